# revision 1
# baseline (speedup 1.0000x reference)
"""Multi-head self-attention (B=2, S=2048, D=1024, H=16, causal) on 8 TRN2 cores.

Sharding: tensor-parallel over heads. Core c owns heads {2c, 2c+1}:
  - Wq/Wk/Wv column-sharded: core c gets columns [128c, 128c+128).
  - All device-side data is bf16 (host casts); psum accumulation f32.
  - Each core computes Q^T,K^T (head-dim on partitions) for its heads via
    matmuls against x^T; V is computed directly token-major by using x^T
    slices as the stationary operand (no PE transposes).
  - Attention in transposed-scores layout: S^T[k, q] tiles, softmax
    denominators from an extra ones-column in V (row 64 of the AV psum).
    Causal masking folded into the scores matmul as a -BIG upper-triangular
    bias matmul on diagonal tiles; fully-masked column blocks are skipped.
  - Normalize Z^T by the per-q reciprocal (gpsimd partition_broadcast).
  - THREE AllToAlls exchange Z^T in bf16: A2A-1 (batch 0, 0.5MB) fires
    mid-phase-A and hides under remaining attention; A2A-2a (batch-1 chunks
    0-1, 0.25MB) fires at the phase-A/B boundary and hides under phase B;
    A2A-2b (batch-1 chunks 2-3, 0.25MB) is the only tail collective and is
    covered by the output projection of the already-received rows plus
    PE-warming dummy matmuls. Core j receives batch-0 tokens
    [256j, 256j+256), batch-1 tokens [128j, 128j+128) and
    [1024+128j, 1024+128j+128); the host reassembles rows accordingly.
  - One PSUM pool scope for the whole body (no cross-phase pool barriers):
    tag "p" (2 bufs) serves projections, phase-B stream-B z, and tail
    dummies; tag "s" (2 bufs) serves fused scores and tail o_ps; tags
    "zA0"/"zA1" serve stream-A z.
  - Output projection per 256-row half: full Wo rows, bias added on DVE.
"""

import ml_dtypes
import numpy as np

import concourse.bass as bass
import concourse.mybir as mybir
import concourse.tile as tile
from concourse import bacc
from concourse.bass_utils import run_bass_kernel_spmd

N_CORES = 8
B, S, D = 2, 2048, 1024
H = 16
HD = D // H          # 64
BS = B * S           # 4096 flattened tokens
CD = 2 * HD          # 128 head-dims per core
NM = S // 512        # 4 q-chunks per batch
BIG = 30000.0
SCALE = 1.0 / np.sqrt(HD)

F32 = mybir.dt.float32
BF16 = mybir.dt.bfloat16
F32R = mybir.dt.float32r
EXP = mybir.ActivationFunctionType.Exp

HEAD_WARM = 36
TAIL_WARM = 40

_CACHE = {}


def build_nc(with_collective=True, reps=1):
    nc = bacc.Bacc("TRN2", target_bir_lowering=False, debug=False, num_devices=N_CORES)

    xT = nc.dram_tensor("xT", [D, BS], BF16, kind="ExternalInput").ap()
    wq = nc.dram_tensor("wq", [128, 8, CD], BF16, kind="ExternalInput").ap()
    wk = nc.dram_tensor("wk", [128, 8, CD], BF16, kind="ExternalInput").ap()
    wv = nc.dram_tensor("wv", [128, 8, CD], BF16, kind="ExternalInput").ap()
    wo = nc.dram_tensor("wo", [128, 8, D], BF16, kind="ExternalInput").ap()
    bo = nc.dram_tensor("bo", [1, D], F32, kind="ExternalInput").ap()
    masku = nc.dram_tensor("masku", [128, 128], BF16, kind="ExternalInput").ap()
    ident = nc.dram_tensor("ident", [128, 128], BF16, kind="ExternalInput").ap()
    ones = nc.dram_tensor("ones", [128, 128], BF16, kind="ExternalInput").ap()
    out = nc.dram_tensor("out", [512, D], F32, kind="ExternalOutput").ap()

    with tile.TileContext(nc) as tc:
        with (
            tc.tile_pool(name="const", bufs=1) as constp,
            tc.tile_pool(name="persist", bufs=1) as persist,
            tc.tile_pool(name="xt", bufs=2) as xtp,
            tc.tile_pool(name="work", bufs=3) as work,
            tc.tile_pool(name="dram", bufs=1, space="DRAM") as dram,
            tc.tile_pool(name="pp", bufs=2, space="PSUM") as ppp,
            tc.tile_pool(name="ps", bufs=2, space="PSUM") as psp,
            tc.tile_pool(name="pz", bufs=1, space="PSUM") as pzp,
        ):
            masku_sb = constp.tile([128, 128], BF16)
            ident_sb = constp.tile([128, 128], BF16)
            ones_sb = constp.tile([128, 128], BF16)
            wsrc = constp.tile([128, 512], BF16)
            cc1_in = dram.tile([8, 128, 256], BF16)
            cc1_out = dram.tile([8, 128, 256], BF16)
            cc2a_in = dram.tile([8, 128, 128], BF16)
            cc2a_out = dram.tile([8, 128, 128], BF16)
            cc2b_in = dram.tile([8, 128, 128], BF16)
            cc2b_out = dram.tile([8, 128, 128], BF16)
            xTr = xT.rearrange("(e p) s -> p e s", p=128)

            for _rep in range(reps):
                _body(nc, tc, constp, persist, xtp, work, dram, ppp, psp, pzp,
                      xTr, wq, wk, wv, wo, bo, out,
                      masku_sb, ident_sb, ones_sb, wsrc,
                      (cc1_in, cc1_out, cc2a_in, cc2a_out, cc2b_in, cc2b_out),
                      with_collective, (masku, ident, ones),
                      first=(_rep == 0))

    nc.compile()
    return nc


def _body(nc, tc, constp, persist, xtp, work, dram, ppp, psp, pzp,
          xTr, wq, wk, wv, wo, bo, out,
          masku_sb, ident_sb, ones_sb, wsrc, cc, with_collective, const_srcs,
          first=True):
    cc1_in, cc1_out, cc2a_in, cc2a_out, cc2b_in, cc2b_out = cc

    # ---- projection weights ----
    wq_sb = constp.tile([128, 8, CD], BF16, tag="wq", name="wq_sb")
    wk_sb = constp.tile([128, 8, CD], BF16, tag="wk", name="wk_sb")
    wv_sb = constp.tile([128, 8, CD], BF16, tag="wv", name="wv_sb")
    if first:
        nc.vector.memset(wsrc[:, 0:128], 0.0)
        nc.gpsimd.memset(wsrc[:, 128:512], 0.0)
    nc.sync.dma_start(wq_sb[:], wq)

    misc = {}

    # ---- persistent activations ----
    qt_sb = persist.tile([128, BS], BF16, tag="qt", name="qt_sb")
    kt_sb = persist.tile([128, BS], BF16, tag="kt", name="kt_sb")
    v_sb = persist.tile([128, 32, 256], BF16, tag="v", name="v_sb")

    def warm(n, name):
        # narrow dummies: depend only on the fast 128-col memset
        ps = ppp.tile([128, 512], F32, tag="p", name=f"warm{name}")
        for i in range(n):
            nc.tensor.matmul(ps[:, 0:128], wsrc[:, 0:128], wsrc[:, 0:128],
                             start=True, stop=True)

    def warm_s(n, name, src):
        # tail dummies: anchored on `src` (the last normalized z tile) so the
        # scheduler cannot hoist them into the attention phase
        ps = psp.tile([128, 512], F32, tag="s", name=f"warm{name}")
        for i in range(n):
            nc.tensor.matmul(ps[:], src[:, 0:128], src[:, 0:512],
                             start=True, stop=True)

    def proj_parts(sc):
        """Yield fine-grained projection closures for one 512-token chunk."""
        sl = bass.ts(sc, 512)
        state = {}

        def load():
            xt_a = xtp.tile([128, 4, 512], BF16, tag="xta", name=f"xta{sc}")
            xt_b = xtp.tile([128, 4, 512], BF16, tag="xtb", name=f"xtb{sc}")
            nc.sync.dma_start(xt_a[:], xTr[:, 0:4, sl])
            nc.sync.dma_start(xt_b[:], xTr[:, 4:8, sl])
            if sc == 0:
                masku_d, ident_d, ones_d = const_srcs
                if first:
                    nc.sync.dma_start(masku_sb[:], masku_d)
                    nc.sync.dma_start(ident_sb[:], ident_d)
                    nc.sync.dma_start(ones_sb[:], ones_d)
                nc.sync.dma_start(wk_sb[:], wk)
                nc.sync.dma_start(wv_sb[:], wv)
                if first:
                    # zero the pad lanes (partitions 1-63 of each AV psum)
                    nc.gpsimd.memset(v_sb[:, :, 1:64], 0.0)
                    nc.gpsimd.memset(v_sb[:, :, 129:192], 0.0)
                    nc.vector.tensor_copy(v_sb[:, :, 0], ones_sb[:, 0:32])
                    nc.vector.tensor_copy(v_sb[:, :, 128], ones_sb[:, 0:32])
            state["xt"] = (xt_a, xt_b)

        def xt_ap(e):
            return state["xt"][e // 4][:, e % 4, :]

        def group(w_sb, o_ap_fn, name):
            def run():
                p_ps = ppp.tile([128, 512], F32, tag="p", name=f"pp{sc}{name}")
                for e in range(8):
                    nc.tensor.matmul(
                        p_ps[:], w_sb[:, e, :], xt_ap(e),
                        start=(e == 0), stop=(e == 7),
                    )
                nc.vector.tensor_copy(o_ap_fn(), p_ps[:])
            return run

        def v_direct(j_range):
            # V token-major: stationary = x^T token slice, moving = Wv slice.
            def run():
                p_ps = state.setdefault(
                    "vp", ppp.tile([128, 512], F32, tag="p", name=f"vp{sc}"))
                for j in j_range:
                    js = slice(128 * j, 128 * j + 128)
                    for e in range(8):
                        nc.tensor.matmul(
                            p_ps[:, js], xt_ap(e)[:, js], wv_sb[:, e, :],
                            start=(e == 0), stop=(e == 7),
                        )
            return run

        def v_copies():
            p_ps = state["vp"]
            for j in range(4):
                tt = 4 * sc + j
                nc.vector.tensor_copy(v_sb[:, tt, 64:128], p_ps[:, 128 * j:128 * j + 64])
                nc.vector.tensor_copy(v_sb[:, tt, 192:256], p_ps[:, 128 * j + 64:128 * j + 128])

        yield load
        yield group(wq_sb, lambda: qt_sb[:, sl], "q")
        yield group(wk_sb, lambda: kt_sb[:, sl], "k")
        yield v_direct(range(0, 2))
        yield v_direct(range(2, 4))
        yield v_copies

    def proj_chunk(sc):
        for part in proj_parts(sc):
            part()

    def attn_chunk_beats(b, m, ztags):
        """Yield one closure per beat; caller weaves streams together."""
        q0 = 2048 * b + 512 * m
        last_t = 4 * m + 3
        state = {}

        def beat(t):
            if t == 0:
                state["z"] = [
                    (pzp if ztags[h].startswith("z") else ppp).tile(
                        [128, 512], F32, tag=ztags[h],
                        name=f"z{b}{m}{h}", bufs=(2 if ztags[h] == "p" else 1))
                    for h in (0, 1)
                ]
            z_ps = state["z"]

            def av(ta, pt_sb):
                joa = max(0, 128 * (ta - 4 * m))
                for h in (0, 1):
                    nc.tensor.matmul(
                        z_ps[h][:, joa:512],
                        v_sb[:, 16 * b + ta, 128 * h:128 * h + 128],
                        pt_sb[:, 512 * h + joa:512 * h + 512],
                        start=(ta == 0), stop=(ta == last_t),
                    )

            k0 = 2048 * b + 128 * t
            jo = max(0, 128 * (t - 4 * m))
            pt_sb = work.tile([128, 1024], BF16, tag="pt", name=f"pt{b}{m}{t}", bufs=6)
            s_ps = psp.tile([128, 1024], F32, tag="s", name=f"s{b}{m}{t}")
            # last two chunks gate the final collective: use the in-psum PE
            # mask there (shorter chain); elsewhere mask on idle gpsimd
            pe_mask = (t >= 4 * m) and b == 1 and m >= 2
            for h in (0, 1):
                hsl = slice(64 * h, 64 * h + 64)
                nc.tensor.matmul(
                    s_ps[:, 512 * h + jo:512 * h + 512],
                    kt_sb[hsl, k0:k0 + 128],
                    qt_sb[hsl, q0 + jo:q0 + 512],
                    start=True, stop=not pe_mask,
                )
                if pe_mask:
                    nc.tensor.matmul(
                        s_ps[:, 512 * h + jo:512 * h + jo + 128],
                        masku_sb[:], ident_sb[:],
                        start=False, stop=True,
                    )
            nc.scalar.activation(
                pt_sb[:].rearrange("p (h w) -> p h w", h=2)[:, :, jo:512],
                s_ps[:].rearrange("p (h w) -> p h w", h=2)[:, :, jo:512],
                EXP, scale=float(SCALE),
            )
            if t >= 4 * m and not pe_mask:
                # causal mask on the diagonal block: keep q >= k, zero rest
                for h in (0, 1):
                    dsl = slice(512 * h + jo, 512 * h + jo + 128)
                    nc.gpsimd.affine_select(
                        out=pt_sb[:, dsl], in_=pt_sb[:, dsl],
                        compare_op=mybir.AluOpType.is_ge,
                        fill=0.0, base=0,
                        pattern=[[1, 128]], channel_multiplier=-1,
                    )
            pend = state.pop("pend", None)
            if pend is not None:
                av(*pend)
            state["pend"] = (t, pt_sb)
            if t == last_t:
                av(*state.pop("pend"))
                _norm(b, m, z_ps)

        for t in range(last_t + 1):
            yield lambda t=t: beat(t)

    def _norm(b, m, z_ps):
        # both reciprocals first (denominators are ready together), then the
        # broadcasts pipeline on gpsimd while DVE normalizes; each batch-1
        # head stages the instant its own multiply lands
        recips, bcs, zts = [], [], []
        for h in (0, 1):
            recip = work.tile([1, 512], F32R, tag="rc", name=f"rc{b}{m}{h}", bufs=2)
            if h == 0:
                misc[f"rc{b}{m}"] = recip
            with nc.allow_low_precision(reason="f32r is bitwise f32 here"):
                nc.vector.reciprocal(recip[0:1, :], z_ps[h][0:1, :].bitcast(F32R))
            recips.append(recip)
        for h in (0, 1):
            bc_sb = work.tile([128, 512], F32R, tag="bc", name=f"bcs{b}{m}{h}", bufs=2)
            nc.gpsimd.partition_broadcast(bc_sb[:], recips[h][0:1, :])
            bcs.append(bc_sb)
        for h in (0, 1):
            zt_h = work.tile([128, 512], BF16, tag=f"zt{h}", name=f"zt{b}{m}{h}", bufs=2)
            nc.vector.tensor_mul(
                zt_h[64:128, :], z_ps[h][64:128, :], bcs[h][64:128, :].bitcast(F32),
            )
            zts.append(zt_h)
            if b == 1:
                cc_in = cc2a_in if m < 2 else cc2b_in
                b0 = 4 * (m % 2)
                nc.sync.dma_start(
                    cc_in[b0:b0 + 4, 64 * h:64 * h + 64, :].rearrange(
                        "k r c -> r k c"),
                    zt_h[64:128, :].rearrange("r (k c) -> r k c", k=4))
        if b == 0:
            for half in (0, 1):
                csl = slice(256 * half, 256 * half + 256)
                nc.sync.dma_start(cc1_in[2 * m + half, 0:64], zts[0][64:128, csl])
                nc.sync.dma_start(cc1_in[2 * m + half, 64:128], zts[1][64:128, csl])

    # ---- issue order ----
    wo_sb = persist.tile([128, 8, D], BF16, tag="wo", name="wo_sb")
    bo_sb = constp.tile([1, D], F32, tag="bo", name="bo_sb")
    bo_bc = constp.tile([128, D], F32, tag="bobc", name="bo_bc")

    def weave(tasks_a, tasks_b, fillers):
        ia = iter(tasks_a)
        ib = iter(tasks_b)
        fi = iter(fillers)
        done_a = done_b = False
        while not (done_a and done_b):
            try:
                next(ia)()
            except StopIteration:
                done_a = True
            try:
                next(ib)()
            except StopIteration:
                done_b = True
            f = next(fi, None)
            if f is not None:
                f()
        for f in fi:
            f()

    def proj_fillers_a():
        for sc in range(1, 8):
            yield from proj_parts(sc)
        yield lambda: nc.sync.dma_start(wo_sb[:], wo)

        def bo_load():
            nc.sync.dma_start(bo_sb[:], bo)
            nc.gpsimd.partition_broadcast(bo_bc[:], bo_sb[:])
        yield bo_load

    def a2a(cin, cout):
        def run():
            if with_collective:
                nc.gpsimd.collective_compute(
                    "AllToAll",
                    mybir.AluOpType.bypass,
                    replica_groups=[list(range(N_CORES))],
                    ins=[cin.opt()],
                    outs=[cout.opt()],
                )
            else:
                nc.sync.dma_start(cout[:], cin[:])
        return run
    a2a_1 = a2a(cc1_in, cc1_out)

    # phase A: projections + batch-0 attention + batch-1 chunks 0-1 (single
    # stream; proj fillers cover chunk boundaries and the A2A-1 issue slots
    # in right after batch-0 is staged)
    warm(HEAD_WARM, "head")
    proj_chunk(0)
    beats_a = (
        list(attn_chunk_beats(0, 0, ("zA0", "zA1")))
        + list(attn_chunk_beats(0, 1, ("zA0", "zA1")))
        + list(attn_chunk_beats(0, 2, ("zA0", "zA1")))
        + list(attn_chunk_beats(0, 3, ("zA0", "zA1")))
        + list(attn_chunk_beats(1, 0, ("zA0", "zA1")))
        + list(attn_chunk_beats(1, 1, ("p", "p")))
    )
    fillers = list(proj_fillers_a())
    fillers.insert(40, a2a_1)
    weave(beats_a, [], fillers)
    a2a(cc2a_in, cc2a_out)()   # (1,0)+(1,1) Z^T, hidden under phase B

    # phase B: remaining batch-1 chunks, two streams ((1,3) borrows the
    # now-idle projection psum ring)
    weave(
        list(attn_chunk_beats(1, 3, ("p", "p"))),
        list(attn_chunk_beats(1, 2, ("zA0", "zA1"))),
        iter(()),
    )

    def wo_st(zt2_ap, st):
        # one 128-row output chunk, cols in 2 halves of 512
        o_sb = work.tile([128, 1024], F32, tag="o", name=f"os{st}", bufs=2)
        for e in range(2):
            o_ps = psp.tile([128, 512], F32, tag="s", name=f"o{st}{e}")
            for i in range(8):
                nc.tensor.matmul(
                    o_ps[:],
                    zt2_ap(i),
                    wo_sb[:, i, bass.ts(e, 512)],
                    start=(i == 0), stop=(i == 7),
                )
            nc.vector.tensor_add(
                o_sb[:, bass.ts(e, 512)], o_ps[:], bo_bc[:, bass.ts(e, 512)])
            nc.sync.dma_start(
                out[bass.ts(st, 128), bass.ts(e, 512)],
                o_sb[:, bass.ts(e, 512)])

    # receives for the two completed exchanges (data ready; no queue blocking)
    zt2_0 = persist.tile([128, 8, 256], BF16, tag="zt20", name="zt2_0")
    ccr1 = cc1_out.rearrange("i p s -> p i s")
    nc.sync.dma_start(zt2_0[:, 0:4, :], ccr1[:, 0:4, :])
    nc.sync.dma_start(zt2_0[:, 4:8, :], ccr1[:, 4:8, :])
    zt2_1a = persist.tile([128, 8, 128], BF16, tag="zt21a", name="zt2_1a")
    ccr2a = cc2a_out.rearrange("i p s -> p i s")
    nc.sync.dma_start(zt2_1a[:, 0:4, :], ccr2a[:, 0:4, :])
    nc.sync.dma_start(zt2_1a[:, 4:8, :], ccr2a[:, 4:8, :])

    # final exchange: only (1,2)+(1,3), 0.25MB
    a2a(cc2b_in, cc2b_out)()
    # 4 independent receive tiles: Wo-st3's first matmuls depend only on the
    # first quarter's DMA, not the whole receive
    ccr2b = cc2b_out.rearrange("i p s -> p i s")
    zt2_1bq = []
    for q in range(4):
        t_q = persist.tile([128, 2, 128], BF16, tag=f"zt21b{q}", name=f"zt2_1b{q}")
        nc.sync.dma_start(t_q[:], ccr2b[:, 2 * q:2 * q + 2, :])
        zt2_1bq.append(t_q)

    # anchored dummies keep the PE p-state hot across the collective wait
    warm_s(TAIL_WARM if with_collective else 8, "tail",
           misc["rc13"].bitcast(BF16)[0:1, :])
    wo_st(lambda i: zt2_0[:, i, 0:128], 0)
    wo_st(lambda i: zt2_0[:, i, 128:256], 1)
    wo_st(lambda i: zt2_1a[:, i, :], 2)
    wo_st(lambda i: zt2_1bq[i // 2][:, i % 2, :], 3)


def _prep_inputs(inputs, Wq, Wk, Wv, Wo, bo):
    bf = ml_dtypes.bfloat16
    x = np.asarray(inputs, dtype=np.float32).reshape(BS, D)
    xT = np.ascontiguousarray(x.T.astype(bf))
    Wq = np.asarray(Wq, dtype=np.float32)
    Wk = np.asarray(Wk, dtype=np.float32)
    Wv = np.asarray(Wv, dtype=np.float32)
    # wo host layout: [p, i, e] = Wo[i*128+p, e]
    wo_h = np.ascontiguousarray(
        np.asarray(Wo, dtype=np.float32).astype(bf).reshape(8, 128, D).transpose(1, 0, 2))
    bo = np.asarray(bo, dtype=np.float32).reshape(1, D)
    masku = np.triu(np.full((128, 128), -BIG, dtype=np.float32), k=1).astype(bf)
    ident = np.eye(128, dtype=np.float32).astype(bf)
    ones = np.ones((128, 128), dtype=np.float32).astype(bf)
    in_maps = []
    for c in range(N_CORES):
        csl = slice(CD * c, CD * (c + 1))

        # weight host layout: [p, e, c] = W[e*128+p, c]
        def wl(W):
            return np.ascontiguousarray(
                W[:, csl].astype(bf).reshape(8, 128, CD).transpose(1, 0, 2))
        in_maps.append({
            "xT": xT,
            "wq": wl(Wq),
            "wk": wl(Wk),
            "wv": wl(Wv),
            "wo": wo_h,
            "bo": bo,
            "masku": masku,
            "ident": ident,
            "ones": ones,
        })
    return in_maps


def kernel(inputs, Wq, Wk, Wv, Wo, bo):
    if "nc" not in _CACHE:
        _CACHE["nc"] = build_nc()
    nc = _CACHE["nc"]
    in_maps = _prep_inputs(inputs, Wq, Wk, Wv, Wo, bo)
    res = None
    for attempt in range(3):
        try:
            res = run_bass_kernel_spmd(nc, in_maps, core_ids=list(range(N_CORES)))
            break
        except Exception:
            if attempt == 2:
                raise
            import time as _time

            _time.sleep(5.0)
    out = np.empty((B, S, D), dtype=np.float32)
    for j in range(N_CORES):
        slab = res.results[j]["out"]
        out[0, 256 * j:256 * j + 256] = slab[0:256]
        out[1, 128 * j:128 * j + 128] = slab[256:384]
        out[1, 1024 + 128 * j:1024 + 128 * j + 128] = slab[384:512]
    return out



# revision 40
# speedup vs baseline: 1.0564x; 1.0564x over previous
"""Multi-head self-attention (B=2, S=2048, D=1024, H=16, causal) on 8 TRN2 cores.

Sharding: tensor-parallel over heads. Core c owns heads {2c, 2c+1}:
  - Wq/Wk/Wv column-sharded: core c gets columns [128c, 128c+128).
  - All device-side data is bf16 (host casts); psum accumulation f32.
  - Each core computes Q^T,K^T (head-dim on partitions) for its heads via
    matmuls against x^T; V is computed directly token-major by using x^T
    slices as the stationary operand (no PE transposes).
  - Scores in transposed layout: S^T[k, q] tiles; softmax denominators from
    a ones-column in the AV moving operand. Causal masking folded into the
    scores matmul as a -BIG upper-triangular bias matmul on diagonal tiles;
    fully-masked column blocks are skipped.
  - AV is token-major (flipped): stationary = exp-scores tile S^T[k, q-tile],
    moving = [ones | V_h] (65 cols) -> z[q, 1+hd] with the denominator in
    col 0. This keeps all 128 output partitions useful (65-row matmuls
    instead of 128-row per (k,q) tile pair), nearly halving AV PE time.
  - Normalize per q-tile on DVE (per-partition reciprocal of col 0, then
    tensor_scalar_mul), pack both heads into [128q, 128d], PE-transpose back
    to z^T[2h*64, 128q] for the collective staging (same wire layout as
    before: head0 rows 0-63, head1 rows 64-127).
  - THREE AllToAlls exchange Z^T in bf16: A2A-1 (batch 0, 0.5MB) fires
    mid-phase-A and hides under remaining attention; A2A-2a (batch-1 chunks
    0-1, 0.25MB) fires at the phase-A/B boundary and hides under phase B;
    A2A-2b (batch-1 chunks 2-3, 0.25MB) is the only tail collective and is
    covered by the output projection of the already-received rows plus
    PE-warming dummy matmuls. Core j receives batch-0 tokens
    [256j, 256j+256), batch-1 tokens [128j, 128j+128) and
    [1024+128j, 1024+128j+128); the host reassembles rows accordingly.
  - PSUM budget (8 banks): "p" ring 2 (projections, phase-B stream-B z,
    head dummies), "s" ring 4 (scores, z transposes, tail o_ps, tail
    dummies), zA0/zA1 2 (stream-A z accumulators). Transposes borrow the
    "s" ring in groups of 4 so the scores double-buffer parity survives.
  - Output projection per 128-row block: full Wo rows, bias added on DVE.
"""

import ml_dtypes
import numpy as np

import concourse.bass as bass
import concourse.mybir as mybir
import concourse.tile as tile
from concourse import bacc
from concourse.bass_utils import run_bass_kernel_spmd

N_CORES = 8
B, S, D = 2, 2048, 1024
H = 16
HD = D // H          # 64
BS = B * S           # 4096 flattened tokens
CD = 2 * HD          # 128 head-dims per core
NM = S // 512        # 4 q-chunks per batch
BIG = 30000.0
SCALE = 1.0 / np.sqrt(HD)

F32 = mybir.dt.float32
BF16 = mybir.dt.bfloat16
F32R = mybir.dt.float32r
EXP = mybir.ActivationFunctionType.Exp

HEAD_WARM = 44
TAIL_WARM = 40

_CACHE = {}


def build_nc(with_collective=True, reps=1):
    nc = bacc.Bacc("TRN2", target_bir_lowering=False, debug=False, num_devices=N_CORES)

    xT = nc.dram_tensor("xT", [D, BS], BF16, kind="ExternalInput").ap()
    wq = nc.dram_tensor("wq", [128, 8, CD], BF16, kind="ExternalInput").ap()
    wk = nc.dram_tensor("wk", [128, 8, CD], BF16, kind="ExternalInput").ap()
    wv = nc.dram_tensor("wv", [128, 8, CD], BF16, kind="ExternalInput").ap()
    wo = nc.dram_tensor("wo", [128, 8, D], BF16, kind="ExternalInput").ap()
    bo = nc.dram_tensor("bo", [1, D], F32, kind="ExternalInput").ap()
    masku = nc.dram_tensor("masku", [128, 128], BF16, kind="ExternalInput").ap()
    ident = nc.dram_tensor("ident", [128, 128], BF16, kind="ExternalInput").ap()
    ones = nc.dram_tensor("ones", [128, 128], BF16, kind="ExternalInput").ap()
    out = nc.dram_tensor("out", [512, D], F32, kind="ExternalOutput").ap()

    with tile.TileContext(nc) as tc:
        with (
            tc.tile_pool(name="const", bufs=1) as constp,
            tc.tile_pool(name="persist", bufs=1) as persist,
            tc.tile_pool(name="xt", bufs=2) as xtp,
            tc.tile_pool(name="work", bufs=3) as work,
            tc.tile_pool(name="dram", bufs=1, space="DRAM") as dram,
            tc.tile_pool(name="pp", bufs=2, space="PSUM") as ppp,
            tc.tile_pool(name="ps", bufs=2, space="PSUM") as psp,
            tc.tile_pool(name="pz", bufs=1, space="PSUM") as pzp,
        ):
            masku_sb = constp.tile([128, 128], BF16)
            ident_sb = constp.tile([128, 128], BF16)
            ones_sb = constp.tile([128, 128], BF16)
            wsrc = constp.tile([128, 512], BF16)
            cc1_in = dram.tile([8, 128, 256], BF16)
            cc1_out = dram.tile([8, 128, 256], BF16)
            cc2a_in = dram.tile([8, 128, 128], BF16)
            cc2a_out = dram.tile([8, 128, 128], BF16)
            cc2b_in = dram.tile([8, 128, 128], BF16)
            cc2b_out = dram.tile([8, 128, 128], BF16)
            xTr = xT.rearrange("(e p) s -> p e s", p=128)

            for _rep in range(reps):
                _body(nc, tc, constp, persist, xtp, work, dram, ppp, psp, pzp,
                      xTr, wq, wk, wv, wo, bo, out,
                      masku_sb, ident_sb, ones_sb, wsrc,
                      (cc1_in, cc1_out, cc2a_in, cc2a_out, cc2b_in, cc2b_out),
                      with_collective, (masku, ident, ones),
                      first=(_rep == 0))

    nc.compile()
    return nc


def _body(nc, tc, constp, persist, xtp, work, dram, ppp, psp, pzp,
          xTr, wq, wk, wv, wo, bo, out,
          masku_sb, ident_sb, ones_sb, wsrc, cc, with_collective, const_srcs,
          first=True):
    cc1_in, cc1_out, cc2a_in, cc2a_out, cc2b_in, cc2b_out = cc

    # ---- projection weights ----
    wq_sb = constp.tile([128, 8, CD], BF16, tag="wq", name="wq_sb")
    wk_sb = constp.tile([128, 8, CD], BF16, tag="wk", name="wk_sb")
    wv_sb = constp.tile([128, 8, CD], BF16, tag="wv", name="wv_sb")
    if first:
        nc.vector.memset(wsrc[:, 0:128], 0.0)
        nc.gpsimd.memset(wsrc[:, 128:512], 0.0)
    nc.sync.dma_start(wq_sb[:], wq)

    misc = {}

    # ---- persistent activations ----
    qt_sb = persist.tile([128, BS], BF16, tag="qt", name="qt_sb")
    kt_sb = persist.tile([128, BS], BF16, tag="kt", name="kt_sb")
    # v tile tt: col 0 = ones (h0 denominator), 1:65 = V_h0, col 65 = ones
    # (h1 denominator), 66:130 = V_h1; partitions = token within 128-block.
    v_sb = persist.tile([128, 32, 130], BF16, tag="v", name="v_sb")

    def warm(n, name):
        # narrow dummies: depend only on the fast 128-col memset
        ps = ppp.tile([128, 512], F32, tag="p", name=f"warm{name}")
        for i in range(n):
            nc.tensor.matmul(ps[:, 0:128], wsrc[:, 0:128], wsrc[:, 0:128],
                             start=True, stop=True)

    def warm_s(n, name, stat_src, mov_src):
        # tail dummies: anchored on the last staged z tiles so the scheduler
        # cannot hoist them into the attention phase
        ps = psp.tile([128, 512], F32, tag="s", name=f"warm{name}")
        for i in range(n):
            nc.tensor.matmul(ps[:], stat_src[:, 0:128], mov_src[:, 0:512],
                             start=True, stop=True)

    def proj_parts(sc):
        """Yield fine-grained projection closures for one 512-token chunk."""
        sl = bass.ts(sc, 512)
        state = {}

        def load():
            xt_a = xtp.tile([128, 4, 512], BF16, tag="xta", name=f"xta{sc}")
            xt_b = xtp.tile([128, 4, 512], BF16, tag="xtb", name=f"xtb{sc}")
            nc.sync.dma_start(xt_a[:], xTr[:, 0:4, sl])
            nc.sync.dma_start(xt_b[:], xTr[:, 4:8, sl])
            if sc == 0:
                masku_d, ident_d, ones_d = const_srcs
                if first:
                    nc.sync.dma_start(masku_sb[:], masku_d)
                    nc.sync.dma_start(ident_sb[:], ident_d)
                    nc.sync.dma_start(ones_sb[:], ones_d)
                nc.sync.dma_start(wk_sb[:], wk)
                nc.sync.dma_start(wv_sb[:], wv)
                if first:
                    # denominator ones-columns for both heads
                    nc.vector.tensor_copy(v_sb[:, :, 0], ones_sb[:, 0:32])
                    nc.vector.tensor_copy(v_sb[:, :, 65], ones_sb[:, 0:32])
            state["xt"] = (xt_a, xt_b)

        def xt_ap(e):
            return state["xt"][e // 4][:, e % 4, :]

        def group(w_sb, o_ap_fn, name):
            def run():
                p_ps = ppp.tile([128, 512], F32, tag="p", name=f"pp{sc}{name}")
                for e in range(8):
                    nc.tensor.matmul(
                        p_ps[:], w_sb[:, e, :], xt_ap(e),
                        start=(e == 0), stop=(e == 7),
                    )
                nc.vector.tensor_copy(o_ap_fn(), p_ps[:])
            return run

        def v_direct(j_range):
            # V token-major: stationary = x^T token slice, moving = Wv slice.
            def run():
                p_ps = state.setdefault(
                    "vp", ppp.tile([128, 512], F32, tag="p", name=f"vp{sc}"))
                for j in j_range:
                    js = slice(128 * j, 128 * j + 128)
                    for e in range(8):
                        nc.tensor.matmul(
                            p_ps[:, js], xt_ap(e)[:, js], wv_sb[:, e, :],
                            start=(e == 0), stop=(e == 7),
                        )
            return run

        def v_copies():
            p_ps = state["vp"]
            for j in range(4):
                tt = 4 * sc + j
                nc.vector.tensor_copy(v_sb[:, tt, 1:65], p_ps[:, 128 * j:128 * j + 64])
                nc.vector.tensor_copy(v_sb[:, tt, 66:130], p_ps[:, 128 * j + 64:128 * j + 128])

        yield load
        yield group(wq_sb, lambda: qt_sb[:, sl], "q")
        yield group(wk_sb, lambda: kt_sb[:, sl], "k")
        yield v_direct(range(0, 2))
        yield v_direct(range(2, 4))
        yield v_copies

    def proj_chunk(sc):
        for part in proj_parts(sc):
            part()

    def attn_chunk(b, m, ztags=None):
        """Return (beat closures, epilogue closures) for one 512-q chunk.

        AV runs as per-(head, q-tile) BURSTS: all k-tiles of one output
        region accumulate consecutively, because PSUM supports only one
        open accumulation group per bank — interleaved groups clobber each
        other. Bursts are issued in the diagonal beats, one beat after the
        last contributing exp; the exp-score tiles stay resident in SBUF.
        """
        q0 = 2048 * b + 512 * m
        last_t = 4 * m + 3
        state = {"pt": {}}

        def burst(qi):
            # one psum tile per q-tile, both heads as sequential groups
            z_ps = pzp.tile([128, 130], F32, tag="z", name=f"z{b}{m}{qi}",
                            bufs=2)
            state[f"z{qi}"] = z_ps
            for h in (0, 1):
                for ta in range(4 * m + qi + 1):
                    nc.tensor.matmul(
                        z_ps[:, 65 * h:65 * h + 65],
                        state["pt"][ta][:, 512 * h + 128 * qi:
                                        512 * h + 128 * qi + 128],
                        v_sb[:, 16 * b + ta, 65 * h:65 * h + 65],
                        start=(ta == 0), stop=(ta == 4 * m + qi),
                    )

        def norm(qn):
            # z_ps[:, 65h] holds the softmax denominator for q-partition
            # rows of q-tile qn; normalize and pack both heads token-major.
            z_ps = state[f"z{qn}"]
            if qn == 0:
                state["rc"] = work.tile([128, 2, 4], F32, tag="rc",
                                        name=f"rc{b}{m}", bufs=2)
                state["zn"] = work.tile([128, 4, 128], BF16, tag="zn",
                                        name=f"zn{b}{m}", bufs=2)
                misc[f"zn{b}{m}"] = state["zn"]
            rc, zn = state["rc"], state["zn"]
            for h in (0, 1):
                nc.vector.reciprocal(rc[:, h, qn:qn + 1],
                                     z_ps[:, 65 * h:65 * h + 1])
            for h in (0, 1):
                nc.vector.tensor_scalar_mul(
                    zn[:, qn, 64 * h:64 * h + 64],
                    z_ps[:, 65 * h + 1:65 * h + 65],
                    rc[:, h, qn:qn + 1],
                )

        def stage(qi):
            # z^T[2h*64, 128q] via PE transpose; same collective wire layout
            # as the direct-z^T scheme (head0 rows 0-63, head1 rows 64-127).
            zn = state["zn"]
            tp = psp.tile([128, 128], BF16, tag="s", name=f"tp{b}{m}{qi}")
            nc.tensor.transpose(tp[:], zn[:, qi, :], ident_sb[:])
            zT = work.tile([128, 128], BF16, tag="zT", name=f"zT{b}{m}{qi}",
                           bufs=4)
            nc.vector.tensor_copy(zT[:], tp[:])
            if b == 0:
                nc.sync.dma_start(
                    cc1_in[2 * m + qi // 2, :,
                           128 * (qi % 2):128 * (qi % 2) + 128],
                    zT[:])
            else:
                cc_in = cc2a_in if m < 2 else cc2b_in
                nc.sync.dma_start(cc_in[4 * (m % 2) + qi, :, :], zT[:])

        def beat(t):
            k0 = 2048 * b + 128 * t
            jo = max(0, 128 * (t - 4 * m))
            pt_sb = work.tile([128, 1024], BF16, tag="pt", name=f"pt{b}{m}{t}", bufs=18)
            s_ps = psp.tile([128, 1024], F32, tag="s", name=f"s{b}{m}{t}")
            # last two chunks gate the final collective: use the in-psum PE
            # mask there (shorter chain); elsewhere mask on idle gpsimd
            pe_mask = (t >= 4 * m) and b == 1 and m >= 2
            for h in (0, 1):
                hsl = slice(64 * h, 64 * h + 64)
                nc.tensor.matmul(
                    s_ps[:, 512 * h + jo:512 * h + 512],
                    kt_sb[hsl, k0:k0 + 128],
                    qt_sb[hsl, q0 + jo:q0 + 512],
                    start=True, stop=not pe_mask,
                )
                if pe_mask:
                    nc.tensor.matmul(
                        s_ps[:, 512 * h + jo:512 * h + jo + 128],
                        masku_sb[:], ident_sb[:],
                        start=False, stop=True,
                    )
            nc.scalar.activation(
                pt_sb[:].rearrange("p (h w) -> p h w", h=2)[:, :, jo:512],
                s_ps[:].rearrange("p (h w) -> p h w", h=2)[:, :, jo:512],
                EXP, scale=float(SCALE),
            )
            if t >= 4 * m and not pe_mask:
                # causal mask on the diagonal block: keep q >= k, zero rest
                for h in (0, 1):
                    dsl = slice(512 * h + jo, 512 * h + jo + 128)
                    nc.gpsimd.affine_select(
                        out=pt_sb[:, dsl], in_=pt_sb[:, dsl],
                        compare_op=mybir.AluOpType.is_ge,
                        fill=0.0, base=0,
                        pattern=[[1, 128]], channel_multiplier=-1,
                    )
            state["pt"][t] = pt_sb
            # burst cadence: beat 4m+1+d runs burst(d)+norm(d) (its last exp
            # landed the previous beat) and stage(d-1); bursts 3 and the
            # final stages spill into the epilogues
            d = t - 4 * m
            if d >= 1:
                burst(d - 1)
                norm(d - 1)
            if d >= 2:
                stage(d - 2)

        def epi1():
            burst(3)
            norm(3)
            stage(2)

        def epi2():
            stage(3)

        beats = [lambda t=t: beat(t) for t in range(last_t + 1)]
        return beats, [epi1, epi2]

    def chain_chunks(chunks, keep_last_epi=True):
        """Flatten chunk beats, delaying each chunk's epilogue closures until
        after the next chunk's second/third beat: the epilogue's burst and
        transpose psum generations must not make the next chunk's first
        scores wait on a DVE queue still draining the chunk-end norms."""
        flat = []
        carry = []
        for bts, epis in chunks:
            flat.extend(bts[:2])
            if carry:
                flat.append(carry[0])
            flat.extend(bts[2:3])
            flat.extend(carry[1:])
            flat.extend(bts[3:])
            carry = epis
        if keep_last_epi:
            flat.extend(carry)
        return flat

    # ---- issue order ----
    wo_sb = persist.tile([128, 8, D], BF16, tag="wo", name="wo_sb")
    bo_sb = constp.tile([1, D], F32, tag="bo", name="bo_sb")
    bo_bc = constp.tile([128, D], F32, tag="bobc", name="bo_bc")

    def weave(tasks_a, tasks_b, fillers):
        ia = iter(tasks_a)
        ib = iter(tasks_b)
        fi = iter(fillers)
        done_a = done_b = False
        while not (done_a and done_b):
            try:
                next(ia)()
            except StopIteration:
                done_a = True
            try:
                next(ib)()
            except StopIteration:
                done_b = True
            f = next(fi, None)
            if f is not None:
                f()
        for f in fi:
            if f is not None:
                f()

    def a2a(cin, cout):
        def run():
            if with_collective:
                nc.gpsimd.collective_compute(
                    "AllToAll",
                    mybir.AluOpType.bypass,
                    replica_groups=[list(range(N_CORES))],
                    ins=[cin.opt()],
                    outs=[cout.opt()],
                )
            else:
                nc.sync.dma_start(cout[:], cin[:])
        return run

    def wo_half_pieces(zt2_ap, st, e, pool, tag, bufs=None, fine=False):
        """Output projection of one 128x512 block, split into 2-matmul
        pieces so woven fillers never starve the Act pipeline. The psum
        pool/tag is chosen by the caller to avoid scores-ring parity."""
        st8 = {}

        def mk(k):
            def run():
                if k == 0:
                    st8["o_ps"] = pool.tile([128, 512], F32, tag=tag,
                                            name=f"o{st}{e}", bufs=bufs)
                for i in (2 * k, 2 * k + 1):
                    nc.tensor.matmul(
                        st8["o_ps"][:], zt2_ap(i), wo_sb[:, i, bass.ts(e, 512)],
                        start=(i == 0), stop=(i == 7),
                    )
            return run

        def fin():
            o_sb = work.tile([128, 512], F32, tag="o", name=f"os{st}{e}", bufs=2)
            for fs in ((slice(0, 256), slice(256, 512)) if fine
                       else (slice(0, 512),)):
                gsl = slice(512 * e + fs.start, 512 * e + fs.stop)
                nc.vector.tensor_add(o_sb[:, fs], st8["o_ps"][:, fs], bo_bc[:, gsl])
                nc.sync.dma_start(out[bass.ts(st, 128), gsl], o_sb[:, fs])
        return [mk(0), mk(1), mk(2), mk(3), fin]

    def wo_st(zt2_ap, st, fine=False):
        for e in (0, 1):
            for piece in wo_half_pieces(zt2_ap, st, e, psp, "s",
                                        fine=(fine and e == 1)):
                piece()

    zt2_0 = persist.tile([128, 8, 256], BF16, tag="zt20", name="zt2_0")
    ccr1 = cc1_out.rearrange("i p s -> p i s")
    zt2_1a = persist.tile([128, 8, 128], BF16, tag="zt21a", name="zt2_1a")
    ccr2a = cc2a_out.rearrange("i p s -> p i s")

    # phase A: projections + batch-0 attention + batch-1 chunks 0-2 (single
    # stream). Proj parts drip through the Act-bound off-diagonal beats with
    # xt loads prefetched two chunks ahead (a filler stalled on its own DMA
    # would block every later PE instruction). Late slots: A2A-1 right after
    # batch-0's staging epilogue, A2A-2a right after (1,1)'s, and the first
    # batch-0 out-projection half once A2A-1 has had ~20us to complete.
    warm(HEAD_WARM, "head")
    proj_chunk(0)
    c12 = attn_chunk(1, 2)
    e12 = c12[1]
    beats_a = chain_chunks([
        attn_chunk(0, 0),
        attn_chunk(0, 1),
        attn_chunk(0, 2),
        attn_chunk(0, 3),
        attn_chunk(1, 0),
        attn_chunk(1, 1),
        c12,
    ], keep_last_epi=False)

    P = {sc: list(proj_parts(sc)) for sc in range(1, 8)}  # L,Q,K,Va,Vb,C

    def wo_load():
        nc.sync.dma_start(wo_sb[:], wo)

    def bo_load():
        nc.sync.dma_start(bo_sb[:], bo)
        nc.gpsimd.partition_broadcast(bo_bc[:], bo_sb[:])

    def rcv1(half):
        def run():
            nc.sync.dma_start(zt2_0[:, 4 * half:4 * half + 4, :],
                              ccr1[:, 4 * half:4 * half + 4, :])
        return run

    def rcv2a(half):
        def run():
            nc.sync.dma_start(zt2_1a[:, 4 * half:4 * half + 4, :],
                              ccr2a[:, 4 * half:4 * half + 4, :])
        return run

    wo00 = wo_half_pieces(lambda i: zt2_0[:, i, 0:128], 0, 0, ppp, "p", bufs=2)
    N = None
    fillers = [
        # 0-12: chunks 1-2 dense (their consumers start at flat 4 / 14)
        P[1][0], P[2][0], P[1][1], P[1][2], P[1][3], P[1][4], P[1][5],
        P[3][0], P[2][1], P[2][2], P[2][3], P[2][4], P[2][5],
        # 13-24: chunk 3 + load 5, thinned
        P[4][0], P[3][1], N, P[3][2], N, P[3][3], N, P[3][4], N, P[3][5],
        N, P[5][0],
        # 25-34: chunk 4 + load 6
        P[4][1], N, P[4][2], P[4][3], N, P[4][4], N, P[4][5], N, P[6][0],
        # 35-43: chunk 5 + load 7 + wo
        P[5][1], N, P[5][2], P[5][3], N, P[5][4], P[5][5], P[7][0], wo_load,
        # 44-49: chunk 6 dense
        bo_load, P[6][1], P[6][2], P[6][3], P[6][4], P[6][5],
        # 50: A2A-1 fires right after epi2(0,3) at flat index 50
        a2a(cc1_in, cc1_out),
        # 51-55: chunk 7 (its qt must land before phase B)
        P[7][1], P[7][2], P[7][3], P[7][4], P[7][5],
        N, N, N, N, N, N, N, N, N, N,
        # 66: A2A-2a fires right after epi2(1,1) at flat index 66; then the
        # batch-0 receives (A2A-1 long done; placed after all staging DMAs
        # so a waiting receive can't head-of-line-block a staging queue)
        a2a(cc2a_in, cc2a_out),
        rcv1(0), rcv1(1),
        # 69-73: first batch-0 out-proj half fills (1,2)'s tail beats
        wo00[0], wo00[1], wo00[2], wo00[3], wo00[4],
    ]
    weave(beats_a, [], fillers)

    # phase B: (1,3) alone; Act-bound beats filled by the remaining batch-0
    # out-projection pieces (psum from the now-idle projection ring)
    b13, e13 = attn_chunk(1, 3)
    wo01 = wo_half_pieces(lambda i: zt2_0[:, i, 0:128], 0, 1, ppp, "p", bufs=2)
    wo10 = wo_half_pieces(lambda i: zt2_0[:, i, 128:256], 1, 0, ppp, "p", bufs=2)
    wo11 = wo_half_pieces(lambda i: zt2_0[:, i, 128:256], 1, 1, ppp, "p", bufs=2)
    fillers_b = [rcv2a(0)] + [e12[0], e12[1]] + wo01 + wo10 + [rcv2a(1)] + wo11
    weave(b13 + e13, [], fillers_b)

    # batch-1a output rows: data received mid-phase-B, keeps PE hot while
    # the final exchange's staging drains
    wo_st(lambda i: zt2_1a[:, i, :], 2)

    # final exchange: only (1,2)+(1,3), 0.25MB
    a2a(cc2b_in, cc2b_out)()
    # 4 independent receive tiles: Wo-st3's first matmuls depend only on the
    # first quarter's DMA, not the whole receive
    ccr2b = cc2b_out.rearrange("i p s -> p i s")
    zt2_1bq = []
    for q in range(4):
        t_q = persist.tile([128, 2, 128], BF16, tag=f"zt21b{q}", name=f"zt2_1b{q}")
        nc.sync.dma_start(t_q[:], ccr2b[:, 2 * q:2 * q + 2, :])
        zt2_1bq.append(t_q)

    # anchored dummies keep the PE p-state hot across the collective wait
    zn13 = misc["zn13"][:].rearrange("p a b -> p (a b)")
    warm_s(TAIL_WARM if with_collective else 18, "tail",
           zn13, zn13)
    wo_st(lambda i: zt2_1bq[i // 2][:, i % 2, :], 3, fine=True)


def _prep_inputs(inputs, Wq, Wk, Wv, Wo, bo):
    bf = ml_dtypes.bfloat16
    x = np.asarray(inputs, dtype=np.float32).reshape(BS, D)
    xT = np.ascontiguousarray(x.T.astype(bf))
    Wq = np.asarray(Wq, dtype=np.float32)
    Wk = np.asarray(Wk, dtype=np.float32)
    Wv = np.asarray(Wv, dtype=np.float32)
    # wo host layout: [p, i, e] = Wo[i*128+p, e]
    wo_h = np.ascontiguousarray(
        np.asarray(Wo, dtype=np.float32).astype(bf).reshape(8, 128, D).transpose(1, 0, 2))
    bo = np.asarray(bo, dtype=np.float32).reshape(1, D)
    masku = np.triu(np.full((128, 128), -BIG, dtype=np.float32), k=1).astype(bf)
    ident = np.eye(128, dtype=np.float32).astype(bf)
    ones = np.ones((128, 128), dtype=np.float32).astype(bf)
    in_maps = []
    for c in range(N_CORES):
        csl = slice(CD * c, CD * (c + 1))

        # weight host layout: [p, e, c] = W[e*128+p, c]
        def wl(W):
            return np.ascontiguousarray(
                W[:, csl].astype(bf).reshape(8, 128, CD).transpose(1, 0, 2))
        in_maps.append({
            "xT": xT,
            "wq": wl(Wq),
            "wk": wl(Wk),
            "wv": wl(Wv),
            "wo": wo_h,
            "bo": bo,
            "masku": masku,
            "ident": ident,
            "ones": ones,
        })
    return in_maps


def kernel(inputs, Wq, Wk, Wv, Wo, bo):
    if "nc" not in _CACHE:
        _CACHE["nc"] = build_nc()
    nc = _CACHE["nc"]
    in_maps = _prep_inputs(inputs, Wq, Wk, Wv, Wo, bo)
    res = None
    for attempt in range(3):
        try:
            res = run_bass_kernel_spmd(nc, in_maps, core_ids=list(range(N_CORES)))
            break
        except Exception:
            if attempt == 2:
                raise
            import time as _time

            _time.sleep(5.0)
    out = np.empty((B, S, D), dtype=np.float32)
    for j in range(N_CORES):
        slab = res.results[j]["out"]
        out[0, 256 * j:256 * j + 256] = slab[0:256]
        out[1, 128 * j:128 * j + 128] = slab[256:384]
        out[1, 1024 + 128 * j:1024 + 128 * j + 128] = slab[384:512]
    return out


# revision 62
# speedup vs baseline: 1.0747x; 1.0173x over previous
"""Multi-head self-attention (B=2, S=2048, D=1024, H=16, causal) on 8 TRN2 cores.

Sharding: tensor-parallel over heads. Core c owns heads {2c, 2c+1}:
  - Wq/Wk/Wv column-sharded: core c gets columns [128c, 128c+128).
  - All device-side data is bf16 (host casts); psum accumulation f32.
  - Each core computes Q^T,K^T (head-dim on partitions) for its heads via
    matmuls against x^T; V is computed directly token-major by using x^T
    slices as the stationary operand (no PE transposes).
  - Scores in transposed layout: S^T[k, q] tiles; softmax denominators from
    a ones-column in the AV moving operand. Causal masking folded into the
    scores matmul as a -BIG upper-triangular bias matmul on diagonal tiles;
    fully-masked column blocks are skipped.
  - AV is token-major (flipped): stationary = exp-scores tile S^T[k, q-tile],
    moving = [ones | V_h] (65 cols) -> z[q, 1+hd] with the denominator in
    col 0. This keeps all 128 output partitions useful (65-row matmuls
    instead of 128-row per (k,q) tile pair), nearly halving AV PE time.
  - Normalize per q-tile on DVE (per-partition reciprocal of col 0, then
    tensor_scalar_mul), pack both heads into [128q, 128d], PE-transpose back
    to z^T[2h*64, 128q] for the collective staging (same wire layout as
    before: head0 rows 0-63, head1 rows 64-127).
  - THREE AllToAlls exchange Z^T in bf16: A2A-1 (batch 0, 0.5MB) fires
    mid-phase-A and hides under remaining attention; A2A-2a (batch-1 chunks
    0-1, 0.25MB) fires at the phase-A/B boundary and hides under phase B;
    A2A-2b (batch-1 chunks 2-3, 0.25MB) is the only tail collective and is
    covered by the output projection of the already-received rows plus
    PE-warming dummy matmuls. Core j receives batch-0 tokens
    [256j, 256j+256), batch-1 tokens [128j, 128j+128) and
    [1024+128j, 1024+128j+128); the host reassembles rows accordingly.
  - PSUM budget (8 banks): "p" ring 2 (projections, phase-B stream-B z,
    head dummies), "s" ring 4 (scores, z transposes, tail o_ps, tail
    dummies), zA0/zA1 2 (stream-A z accumulators). Transposes borrow the
    "s" ring in groups of 4 so the scores double-buffer parity survives.
  - Output projection per 128-row block: full Wo rows, bias added on DVE.
"""

import ml_dtypes
import numpy as np

import concourse.bass as bass
import concourse.mybir as mybir
import concourse.tile as tile
from concourse import bacc
from concourse.bass_utils import run_bass_kernel_spmd

N_CORES = 8
B, S, D = 2, 2048, 1024
H = 16
HD = D // H          # 64
BS = B * S           # 4096 flattened tokens
CD = 2 * HD          # 128 head-dims per core
NM = S // 512        # 4 q-chunks per batch
BIG = 30000.0
SCALE = 1.0 / np.sqrt(HD)

F32 = mybir.dt.float32
BF16 = mybir.dt.bfloat16
F32R = mybir.dt.float32r
EXP = mybir.ActivationFunctionType.Exp

HEAD_WARM = 34
TAIL_WARM = 48

_CACHE = {}


def build_nc(with_collective=True, reps=1):
    nc = bacc.Bacc("TRN2", target_bir_lowering=False, debug=False, num_devices=N_CORES)

    xT = nc.dram_tensor("xT", [D, BS], BF16, kind="ExternalInput").ap()
    wq = nc.dram_tensor("wq", [128, 8, CD], BF16, kind="ExternalInput").ap()
    wk = nc.dram_tensor("wk", [128, 8, CD], BF16, kind="ExternalInput").ap()
    wv = nc.dram_tensor("wv", [128, 8, CD], BF16, kind="ExternalInput").ap()
    wo = nc.dram_tensor("wo", [128, 8, D], BF16, kind="ExternalInput").ap()
    bo = nc.dram_tensor("bo", [1, D], F32, kind="ExternalInput").ap()
    masku = nc.dram_tensor("masku", [128, 128], BF16, kind="ExternalInput").ap()
    ident = nc.dram_tensor("ident", [128, 128], BF16, kind="ExternalInput").ap()
    ones = nc.dram_tensor("ones", [128, 128], BF16, kind="ExternalInput").ap()
    out = nc.dram_tensor("out", [512, D], F32, kind="ExternalOutput").ap()

    with tile.TileContext(nc) as tc:
        with (
            tc.tile_pool(name="const", bufs=1) as constp,
            tc.tile_pool(name="persist", bufs=1) as persist,
            tc.tile_pool(name="xt", bufs=2) as xtp,
            tc.tile_pool(name="work", bufs=3) as work,
            tc.tile_pool(name="dram", bufs=1, space="DRAM") as dram,
            tc.tile_pool(name="pp", bufs=2, space="PSUM") as ppp,
            tc.tile_pool(name="ps", bufs=2, space="PSUM") as psp,
            tc.tile_pool(name="pz", bufs=1, space="PSUM") as pzp,
        ):
            masku_sb = constp.tile([128, 128], BF16)
            ident_sb = constp.tile([128, 128], BF16)
            ones_sb = constp.tile([128, 128], BF16)
            wsrc = constp.tile([128, 512], BF16)
            cc1_in = dram.tile([8, 128, 256], BF16)
            cc1_out = dram.tile([8, 128, 256], BF16)
            cc2a_in = dram.tile([8, 128, 128], BF16)
            cc2a_out = dram.tile([8, 128, 128], BF16)
            cc2b_in = dram.tile([8, 128, 128], BF16)
            cc2b_out = dram.tile([8, 128, 128], BF16)
            xTr = xT.rearrange("(e p) s -> p e s", p=128)

            for _rep in range(reps):
                _body(nc, tc, constp, persist, xtp, work, dram, ppp, psp, pzp,
                      xTr, wq, wk, wv, wo, bo, out,
                      masku_sb, ident_sb, ones_sb, wsrc,
                      (cc1_in, cc1_out, cc2a_in, cc2a_out, cc2b_in, cc2b_out),
                      with_collective, (masku, ident, ones),
                      first=(_rep == 0))

    nc.compile()
    return nc


def _body(nc, tc, constp, persist, xtp, work, dram, ppp, psp, pzp,
          xTr, wq, wk, wv, wo, bo, out,
          masku_sb, ident_sb, ones_sb, wsrc, cc, with_collective, const_srcs,
          first=True):
    cc1_in, cc1_out, cc2a_in, cc2a_out, cc2b_in, cc2b_out = cc

    # ---- projection weights ----
    wq_sb = constp.tile([128, 8, CD], BF16, tag="wq", name="wq_sb")
    wk_sb = constp.tile([128, 8, CD], BF16, tag="wk", name="wk_sb")
    wv_sb = constp.tile([128, 8, CD], BF16, tag="wv", name="wv_sb")
    if first:
        # warm dummies read cols 0:128 — zero those on the fast-starting Pool
        nc.gpsimd.memset(wsrc[:, 0:128], 0.0)
        nc.vector.memset(wsrc[:, 128:512], 0.0)
    nc.sync.dma_start(wq_sb[:], wq)

    misc = {}

    # ---- persistent activations ----
    qt_sb = persist.tile([128, BS], BF16, tag="qt", name="qt_sb")
    kt_sb = persist.tile([128, BS], BF16, tag="kt", name="kt_sb")
    # v tile tt: col 0 = ones (h0 denominator), 1:65 = V_h0, col 65 = ones
    # (h1 denominator), 66:130 = V_h1; partitions = token within 128-block.
    v_sb = persist.tile([128, 32, 130], BF16, tag="v", name="v_sb")

    def warm(n, name):
        # narrow dummies: depend only on the fast 128-col memset
        ps = ppp.tile([128, 512], F32, tag="p", name=f"warm{name}")
        for i in range(n):
            nc.tensor.matmul(ps[:, 0:128], wsrc[:, 0:128], wsrc[:, 0:128],
                             start=True, stop=True)

    def warm_s(n, name, stat_src, mov_src):
        # tail dummies: anchored on the last staged z tiles so the scheduler
        # cannot hoist them into the attention phase
        ps = psp.tile([128, 512], F32, tag="s", name=f"warm{name}")
        for i in range(n):
            nc.tensor.matmul(ps[:], stat_src[:, 0:128], mov_src[:, 0:512],
                             start=True, stop=True)

    def proj_parts(sc):
        """Yield fine-grained projection closures for one 512-token chunk."""
        sl = bass.ts(sc, 512)
        state = {}

        def load():
            xt_a = xtp.tile([128, 4, 512], BF16, tag="xta", name=f"xta{sc}")
            xt_b = xtp.tile([128, 4, 512], BF16, tag="xtb", name=f"xtb{sc}")
            nc.sync.dma_start(xt_a[:], xTr[:, 0:4, sl])
            nc.sync.dma_start(xt_b[:], xTr[:, 4:8, sl])
            if sc == 0:
                masku_d, ident_d, ones_d = const_srcs
                if first:
                    nc.sync.dma_start(masku_sb[:], masku_d)
                    nc.sync.dma_start(ident_sb[:], ident_d)
                    nc.sync.dma_start(ones_sb[:], ones_d)
                nc.sync.dma_start(wk_sb[:], wk)
                nc.sync.dma_start(wv_sb[:], wv)
                if first:
                    # denominator ones-columns for both heads
                    nc.vector.tensor_copy(v_sb[:, :, 0], ones_sb[:, 0:32])
                    nc.vector.tensor_copy(v_sb[:, :, 65], ones_sb[:, 0:32])
            state["xt"] = (xt_a, xt_b)

        def xt_ap(e):
            return state["xt"][e // 4][:, e % 4, :]

        def group(w_sb, o_ap_fn, name):
            def run():
                p_ps = ppp.tile([128, 512], F32, tag="p", name=f"pp{sc}{name}")
                for e in range(8):
                    nc.tensor.matmul(
                        p_ps[:], w_sb[:, e, :], xt_ap(e),
                        start=(e == 0), stop=(e == 7),
                    )
                nc.vector.tensor_copy(o_ap_fn(), p_ps[:])
            return run

        def v_direct(j_range):
            # V token-major: stationary = x^T token slice, moving = Wv slice.
            def run():
                p_ps = state.setdefault(
                    "vp", ppp.tile([128, 512], F32, tag="p", name=f"vp{sc}"))
                for j in j_range:
                    js = slice(128 * j, 128 * j + 128)
                    for e in range(8):
                        nc.tensor.matmul(
                            p_ps[:, js], xt_ap(e)[:, js], wv_sb[:, e, :],
                            start=(e == 0), stop=(e == 7),
                        )
            return run

        def v_copies(j_range=range(4)):
            p_ps = state["vp"]
            for j in j_range:
                tt = 4 * sc + j
                nc.vector.tensor_copy(v_sb[:, tt, 1:65], p_ps[:, 128 * j:128 * j + 64])
                nc.vector.tensor_copy(v_sb[:, tt, 66:130], p_ps[:, 128 * j + 64:128 * j + 128])

        yield load
        yield group(wq_sb, lambda: qt_sb[:, sl], "q")
        yield group(wk_sb, lambda: kt_sb[:, sl], "k")
        yield v_direct(range(0, 2))
        yield v_direct(range(2, 4))
        yield v_copies

    def proj_chunk(sc):
        for part in proj_parts(sc):
            part()

    def attn_chunk(b, m, ztags=None):
        """Return (beat closures, epilogue closures) for one 512-q chunk.

        AV runs as per-(head, q-tile) BURSTS: all k-tiles of one output
        region accumulate consecutively, because PSUM supports only one
        open accumulation group per bank — interleaved groups clobber each
        other. Bursts are issued in the diagonal beats, one beat after the
        last contributing exp; the exp-score tiles stay resident in SBUF.
        """
        q0 = 2048 * b + 512 * m
        last_t = 4 * m + 3
        state = {"pt": {}}

        def burst(qi):
            # one psum tile per q-tile, both heads as sequential groups
            z_ps = pzp.tile([128, 130], F32, tag="z", name=f"z{b}{m}{qi}",
                            bufs=2)
            state[f"z{qi}"] = z_ps
            for h in (0, 1):
                for ta in range(4 * m + qi + 1):
                    nc.tensor.matmul(
                        z_ps[:, 65 * h:65 * h + 65],
                        state["pt"][ta][:, 512 * h + 128 * qi:
                                        512 * h + 128 * qi + 128],
                        v_sb[:, 16 * b + ta, 65 * h:65 * h + 65],
                        start=(ta == 0), stop=(ta == 4 * m + qi),
                    )

        def norm(qn):
            # z_ps[:, 65h] holds the softmax denominator for q-partition
            # rows of q-tile qn; normalize and pack both heads token-major.
            z_ps = state[f"z{qn}"]
            if qn == 0:
                state["rc"] = work.tile([128, 2, 4], F32, tag="rc",
                                        name=f"rc{b}{m}", bufs=2)
                state["zn"] = work.tile([128, 4, 128], BF16, tag="zn",
                                        name=f"zn{b}{m}", bufs=2)
                misc[f"zn{b}{m}"] = state["zn"]
            rc, zn = state["rc"], state["zn"]
            for h in (0, 1):
                nc.vector.reciprocal(rc[:, h, qn:qn + 1],
                                     z_ps[:, 65 * h:65 * h + 1])
            for h in (0, 1):
                nc.vector.tensor_scalar_mul(
                    zn[:, qn, 64 * h:64 * h + 64],
                    z_ps[:, 65 * h + 1:65 * h + 65],
                    rc[:, h, qn:qn + 1],
                )

        def stage(qi):
            # z^T[2h*64, 128q] via PE transpose; same collective wire layout
            # as the direct-z^T scheme (head0 rows 0-63, head1 rows 64-127).
            zn = state["zn"]
            # tp lives in the z-ring: scores generations must never wait on
            # a transpose's staging-DMA release. The z-ring cross-waits are
            # cheap (burst release = norm read, tp release = staging DMA),
            # provided no receive DMA can head-of-line-block the staging
            # queue (receives are placed after the exchanges complete).
            tp = pzp.tile([128, 128], BF16, tag="z", name=f"tp{b}{m}{qi}",
                          bufs=2)
            nc.tensor.transpose(tp[:], zn[:, qi, :], ident_sb[:])
            zT = work.tile([128, 128], BF16, tag="zT", name=f"zT{b}{m}{qi}",
                           bufs=4)
            nc.vector.tensor_copy(zT[:], tp[:])
            if b == 0:
                nc.sync.dma_start(
                    cc1_in[2 * m + qi // 2, :,
                           128 * (qi % 2):128 * (qi % 2) + 128],
                    zT[:])
            else:
                cc_in = cc2a_in if m < 2 else cc2b_in
                nc.sync.dma_start(cc_in[4 * (m % 2) + qi, :, :], zT[:])

        def beat(t):
            k0 = 2048 * b + 128 * t
            jo = max(0, 128 * (t - 4 * m))
            pt_sb = work.tile([128, 1024], BF16, tag="pt", name=f"pt{b}{m}{t}", bufs=18)
            s_ps = psp.tile([128, 1024], F32, tag="s", name=f"s{b}{m}{t}")
            # last two chunks gate the final collective: use the in-psum PE
            # mask there (shorter chain); elsewhere mask on idle gpsimd
            pe_mask = (t >= 4 * m) and b == 1 and m >= 2
            for h in (0, 1):
                hsl = slice(64 * h, 64 * h + 64)
                nc.tensor.matmul(
                    s_ps[:, 512 * h + jo:512 * h + 512],
                    kt_sb[hsl, k0:k0 + 128],
                    qt_sb[hsl, q0 + jo:q0 + 512],
                    start=True, stop=not pe_mask,
                )
                if pe_mask:
                    nc.tensor.matmul(
                        s_ps[:, 512 * h + jo:512 * h + jo + 128],
                        masku_sb[:], ident_sb[:],
                        start=False, stop=True,
                    )
            nc.scalar.activation(
                pt_sb[:].rearrange("p (h w) -> p h w", h=2)[:, :, jo:512],
                s_ps[:].rearrange("p (h w) -> p h w", h=2)[:, :, jo:512],
                EXP, scale=float(SCALE),
            )
            if t >= 4 * m and not pe_mask:
                # causal mask on the diagonal block: keep q >= k, zero rest
                for h in (0, 1):
                    dsl = slice(512 * h + jo, 512 * h + jo + 128)
                    nc.gpsimd.affine_select(
                        out=pt_sb[:, dsl], in_=pt_sb[:, dsl],
                        compare_op=mybir.AluOpType.is_ge,
                        fill=0.0, base=0,
                        pattern=[[1, 128]], channel_multiplier=-1,
                    )
            state["pt"][t] = pt_sb
            # burst cadence: beat 4m+1+d runs burst(d)+norm(d) (its last exp
            # landed the previous beat); stages run as parity-safe PAIRS:
            # 0+1 at the last beat, 2+3 in the second epilogue
            d = t - 4 * m
            if d >= 1:
                burst(d - 1)
                norm(d - 1)
            if d == 3:
                stage(0)
                stage(1)

        def epi1():
            burst(3)
            norm(3)

        def epi2():
            stage(2)
            stage(3)

        beats = [lambda t=t: beat(t) for t in range(last_t + 1)]
        return beats, [epi1, epi2]

    def chain_chunks(chunks, keep_last_epi=True):
        """Flatten chunk beats, delaying each chunk's epilogue closures until
        after the next chunk's second/third beat: the epilogue's burst and
        transpose psum generations must not make the next chunk's first
        scores wait on a DVE queue still draining the chunk-end norms."""
        flat = []
        carry = []
        for bts, epis in chunks:
            flat.extend(bts[:2])
            if carry:
                flat.append(carry[0])
            flat.extend(bts[2:3])
            flat.extend(carry[1:])
            flat.extend(bts[3:])
            carry = epis
        if keep_last_epi:
            flat.extend(carry)
        return flat

    # ---- issue order ----
    wo_sb = persist.tile([128, 8, D], BF16, tag="wo", name="wo_sb")
    bo_sb = constp.tile([1, D], F32, tag="bo", name="bo_sb")
    bo_bc = constp.tile([128, D], F32, tag="bobc", name="bo_bc")

    def weave(tasks_a, tasks_b, fillers):
        ia = iter(tasks_a)
        ib = iter(tasks_b)
        fi = iter(fillers)
        done_a = done_b = False
        while not (done_a and done_b):
            try:
                next(ia)()
            except StopIteration:
                done_a = True
            try:
                next(ib)()
            except StopIteration:
                done_b = True
            f = next(fi, None)
            if f is not None:
                f()
        for f in fi:
            if f is not None:
                f()

    def a2a(cin, cout):
        def run():
            if with_collective:
                nc.gpsimd.collective_compute(
                    "AllToAll",
                    mybir.AluOpType.bypass,
                    replica_groups=[list(range(N_CORES))],
                    ins=[cin.opt()],
                    outs=[cout.opt()],
                )
            else:
                nc.sync.dma_start(cout[:], cin[:])
        return run

    def wo_half_pieces(zt2_ap, st, e, pool, tag, bufs=None, fine=False):
        """Output projection of one 128x512 block, split into 2-matmul
        pieces so woven fillers never starve the Act pipeline. The psum
        pool/tag is chosen by the caller to avoid scores-ring parity."""
        st8 = {}

        def mk(k):
            def run():
                if k == 0:
                    st8["o_ps"] = pool.tile([128, 512], F32, tag=tag,
                                            name=f"o{st}{e}", bufs=bufs)
                for i in (2 * k, 2 * k + 1):
                    nc.tensor.matmul(
                        st8["o_ps"][:], zt2_ap(i), wo_sb[:, i, bass.ts(e, 512)],
                        start=(i == 0), stop=(i == 7),
                    )
            return run

        def fin():
            o_sb = work.tile([128, 512], F32, tag="o", name=f"os{st}{e}", bufs=2)
            for fs in ((slice(0, 256), slice(256, 512)) if fine
                       else (slice(0, 512),)):
                gsl = slice(512 * e + fs.start, 512 * e + fs.stop)
                nc.vector.tensor_add(o_sb[:, fs], st8["o_ps"][:, fs], bo_bc[:, gsl])
                nc.sync.dma_start(out[bass.ts(st, 128), gsl], o_sb[:, fs])
        return [mk(0), mk(1), mk(2), mk(3), fin]

    def wo_st(zt2_ap, st, fine=False):
        for e in (0, 1):
            for piece in wo_half_pieces(zt2_ap, st, e, psp, "s",
                                        fine=(fine and e == 1)):
                piece()

    zt2_0 = persist.tile([128, 8, 256], BF16, tag="zt20", name="zt2_0")
    ccr1 = cc1_out.rearrange("i p s -> p i s")
    zt2_1a = persist.tile([128, 8, 128], BF16, tag="zt21a", name="zt2_1a")
    ccr2a = cc2a_out.rearrange("i p s -> p i s")

    # phase A: projections + batch-0 attention + batch-1 chunks 0-2 (single
    # stream). Proj parts drip through the Act-bound off-diagonal beats with
    # xt loads prefetched two chunks ahead (a filler stalled on its own DMA
    # would block every later PE instruction). Late slots: A2A-1 right after
    # batch-0's staging epilogue, A2A-2a right after (1,1)'s, and the first
    # batch-0 out-projection half once A2A-1 has had ~20us to complete.
    # chunk 0: only V-tiles 0-1 are needed before beat 3; tiles 2-3 move
    # into the first filler so beat 0 starts ~1.5us earlier
    P0 = list(proj_parts(0))
    warm(HEAD_WARM, "head")
    for part in P0[:4]:
        part()
    P0[5](range(0, 2))

    def p0_rest():
        P0[4]()
        P0[5](range(2, 4))
    c12 = attn_chunk(1, 2)
    e12 = c12[1]
    beats_a = chain_chunks([
        attn_chunk(0, 0),
        attn_chunk(0, 1),
        attn_chunk(0, 2),
        attn_chunk(0, 3),
        attn_chunk(1, 0),
        attn_chunk(1, 1),
        c12,
    ], keep_last_epi=False)

    P = {sc: list(proj_parts(sc)) for sc in range(1, 8)}  # L,Q,K,Va,Vb,C

    def wo_load():
        nc.sync.dma_start(wo_sb[:], wo)

    def bo_load():
        nc.sync.dma_start(bo_sb[:], bo)
        nc.gpsimd.partition_broadcast(bo_bc[:], bo_sb[:])

    def rcv1(half):
        def run():
            nc.sync.dma_start(zt2_0[:, 4 * half:4 * half + 4, :],
                              ccr1[:, 4 * half:4 * half + 4, :])
        return run

    def rcv2a(half):
        def run():
            nc.sync.dma_start(zt2_1a[:, 4 * half:4 * half + 4, :],
                              ccr2a[:, 4 * half:4 * half + 4, :])
        return run

    wo00 = wo_half_pieces(lambda i: zt2_0[:, i, 0:128], 0, 0, ppp, "p", bufs=2)
    N = None

    def f0():
        p0_rest()
        P[1][0]()
    fillers = [
        # 0-12: chunks 1-2 dense (their consumers start at flat 4 / 14)
        f0, P[2][0], P[1][1], P[1][2], P[1][3], P[1][4], P[1][5],
        P[3][0], P[2][1], P[2][2], P[2][3], P[2][4], P[2][5],
        # 13-24: chunk 3 + load 5, thinned
        P[4][0], P[3][1], N, P[3][2], N, P[3][3], N, P[3][4], N, P[3][5],
        N, P[5][0],
        # 25-34: chunk 4 + load 6
        P[4][1], N, P[4][2], P[4][3], N, P[4][4], N, P[4][5], N, P[6][0],
        # 35-43: chunk 5 + load 7 + wo
        P[5][1], N, P[5][2], P[5][3], N, P[5][4], P[5][5], P[7][0], wo_load,
        # 44-49: (1,0) is all-diagonal (burst-heavy, Act-light): no fill
        bo_load, N, N, N, N, N,
        # 50: A2A-1 fires right after epi2(0,3) at flat index 50
        a2a(cc1_in, cc1_out),
        # 51-60: chunks 6-7 fill (1,1)'s off-diagonal beats; C6 lands just
        # before (1,2) needs qt6 at flat 62, C7 well before phase B
        P[6][1], P[6][2], P[6][3], P[6][4], P[6][5],
        P[7][1], P[7][2], P[7][3], P[7][4], P[7][5],
        N, N, N, N, N,
        # 66: A2A-2a fires right after epi2(1,1) at flat index 66; then the
        # batch-0 receives (A2A-1 long done; placed after all staging DMAs
        # so a waiting receive can't head-of-line-block a staging queue)
        a2a(cc2a_in, cc2a_out),
        rcv1(0), rcv1(1),
        # 69-73: first batch-0 out-proj half fills (1,2)'s tail beats
        wo00[0], wo00[1], wo00[2], wo00[3], wo00[4],
    ]
    weave(beats_a, [], fillers)

    # phase B: (1,3) alone; Act-bound beats filled by the remaining batch-0
    # out-projection pieces (psum from the now-idle projection ring).
    b13, e13 = attn_chunk(1, 3)
    wo01 = wo_half_pieces(lambda i: zt2_0[:, i, 0:128], 0, 1, ppp, "p", bufs=2)
    wo10 = wo_half_pieces(lambda i: zt2_0[:, i, 128:256], 1, 0, ppp, "p", bufs=2)
    wo11 = wo_half_pieces(lambda i: zt2_0[:, i, 128:256], 1, 1, ppp, "p", bufs=2)
    # st2's first half also weaves in (A2A-2a has had ~16us by then), paired
    # with wo11 pieces — the two o_ps generations sit in the p-ring's two
    # banks, so their accumulation groups never share a bank
    st2e0 = wo_half_pieces(lambda i: zt2_1a[:, i, :], 2, 0, ppp, "p", bufs=2)

    def pair(x, y):
        def run():
            x()
            y()
        return run
    fillers_b = ([rcv2a(0), e12[0], e12[1]]
                 + wo01 + wo10 + [rcv2a(1)]
                 + [pair(x, y) for x, y in zip(wo11, st2e0)])
    weave(b13 + e13, [], fillers_b)

    # batch-1a output rows (second half; the first wove into phase B):
    # data received mid-phase-B, keeps PE hot while the final exchange's
    # staging drains
    for piece in wo_half_pieces(lambda i: zt2_1a[:, i, :], 2, 1, psp, "s"):
        piece()

    # final exchange: only (1,2)+(1,3), 0.25MB
    a2a(cc2b_in, cc2b_out)()
    # 4 independent receive tiles: Wo-st3's first matmuls depend only on the
    # first quarter's DMA, not the whole receive
    ccr2b = cc2b_out.rearrange("i p s -> p i s")
    zt2_1bq = []
    for q in range(4):
        t_q = persist.tile([128, 2, 128], BF16, tag=f"zt21b{q}", name=f"zt2_1b{q}")
        nc.sync.dma_start(t_q[:], ccr2b[:, 2 * q:2 * q + 2, :])
        zt2_1bq.append(t_q)

    # anchored dummies keep the PE p-state hot across the collective wait
    zn13 = misc["zn13"][:].rearrange("p a b -> p (a b)")
    warm_s(TAIL_WARM if with_collective else 18, "tail",
           zn13, zn13)
    wo_st(lambda i: zt2_1bq[i // 2][:, i % 2, :], 3, fine=True)


def _prep_inputs(inputs, Wq, Wk, Wv, Wo, bo):
    bf = ml_dtypes.bfloat16
    x = np.asarray(inputs, dtype=np.float32).reshape(BS, D)
    xT = np.ascontiguousarray(x.T.astype(bf))
    Wq = np.asarray(Wq, dtype=np.float32)
    Wk = np.asarray(Wk, dtype=np.float32)
    Wv = np.asarray(Wv, dtype=np.float32)
    # wo host layout: [p, i, e] = Wo[i*128+p, e]
    wo_h = np.ascontiguousarray(
        np.asarray(Wo, dtype=np.float32).astype(bf).reshape(8, 128, D).transpose(1, 0, 2))
    bo = np.asarray(bo, dtype=np.float32).reshape(1, D)
    masku = np.triu(np.full((128, 128), -BIG, dtype=np.float32), k=1).astype(bf)
    ident = np.eye(128, dtype=np.float32).astype(bf)
    ones = np.ones((128, 128), dtype=np.float32).astype(bf)
    in_maps = []
    for c in range(N_CORES):
        csl = slice(CD * c, CD * (c + 1))

        # weight host layout: [p, e, c] = W[e*128+p, c]
        def wl(W):
            return np.ascontiguousarray(
                W[:, csl].astype(bf).reshape(8, 128, CD).transpose(1, 0, 2))
        in_maps.append({
            "xT": xT,
            "wq": wl(Wq),
            "wk": wl(Wk),
            "wv": wl(Wv),
            "wo": wo_h,
            "bo": bo,
            "masku": masku,
            "ident": ident,
            "ones": ones,
        })
    return in_maps


def kernel(inputs, Wq, Wk, Wv, Wo, bo):
    if "nc" not in _CACHE:
        _CACHE["nc"] = build_nc()
    nc = _CACHE["nc"]
    in_maps = _prep_inputs(inputs, Wq, Wk, Wv, Wo, bo)
    res = None
    for attempt in range(3):
        try:
            res = run_bass_kernel_spmd(nc, in_maps, core_ids=list(range(N_CORES)))
            break
        except Exception:
            if attempt == 2:
                raise
            import time as _time

            _time.sleep(5.0)
    out = np.empty((B, S, D), dtype=np.float32)
    for j in range(N_CORES):
        slab = res.results[j]["out"]
        out[0, 256 * j:256 * j + 256] = slab[0:256]
        out[1, 128 * j:128 * j + 128] = slab[256:384]
        out[1, 1024 + 128 * j:1024 + 128 * j + 128] = slab[384:512]
    return out
